# revision 23
# baseline (speedup 1.0000x reference)
"""Multi-head attention (B=4, S=2048, d_model=1024, 16 heads x 64) on 8 trn2 cores.

Sharding: tensor-parallel over heads -- each core owns 2 heads (128 of the
1024 q/k/v dims and 128 columns of Wo's input dim). Each core computes a
partial output projection yT_c [1024, 8192]; the host sums the 8 partials,
adds bo, and transposes back to [4, 2048, 1024].

v2 redesign vs the fp32r baseline (977 us):
- All matmul operands bf16: separate LDWEIGHTS (background-buffer
  pull-ahead + automatic FWL for 128-col weights) instead of fp32r's
  self-loading matmuls that serialize weight load with compute.
- The two heads' score matmuls (contraction 64) are issued back to back
  on disjoint partition halves, so the PE runs them concurrently via
  row-group tiling (base_partition 0 / 64 -> tile_position rows 0 / 64).
- V transposed by DMA (dma_start_transpose) instead of PE+DVE.
- Next batch's q/k/v projections and previous output-projection chunks
  are interleaved into the attention loop as fillers, keeping the PE
  continuously busy so the HAM clock gate stays at 2.4 GHz.
- Softmax normalization per 512-q chunk: denominators ride along the AV
  matmul via a ones column in V; reciprocal_approx_fast (5x cheaper than
  DVE reciprocal), rank-1 PE broadcast, normalize fused into the
  PSUM->SBUF evacuation of the attention output.
"""

from collections import deque

import numpy as np
import ml_dtypes

import concourse.bass as bass
import concourse.mybir as mybir
from concourse import bacc
from concourse.tile import TileContext
from concourse.bass_utils import run_bass_kernel_spmd

N_HEAD = 16
D_HEAD = 64
D_MODEL = N_HEAD * D_HEAD  # 1024
B, S = 4, 2048
N_CORES = 8
HPC = N_HEAD // N_CORES  # heads per core = 2
HD = HPC * D_HEAD        # per-core head dims = 128

F32 = mybir.dt.float32
F32R = mybir.dt.float32r
BF16 = mybir.dt.bfloat16
AF = mybir.ActivationFunctionType
BF16_NP = ml_dtypes.bfloat16

_TRACE = False  # test harness can flip this for profiling
_DMA_VTRANS = True  # False: PE-transpose fallback for the V relayout
_INTERLEAVE = True  # False: run fillers en-bloc at batch boundaries
_DEBUG_DUMP = False  # True: DMA per-stage intermediates for batches 0/1


def build_mha(b=B, s=S, dm=D_MODEL, hd=HD, d=D_HEAD):
    """Build the per-core Bass program (SPMD; all cores run this)."""
    P = 128
    tok = b * s
    dmc = dm // P            # contraction chunks for projections (8)
    n_tc = s // 512          # 512-token proj chunks per batch (4)
    n_kt = s // P            # 128-token k tiles per batch (16)
    n_qc = s // 512          # 512-token q chunks per batch (4)
    hpc = hd // d            # heads per core (2)
    vstr = 80                # v65 chunk stride (160B, 32B-aligned for xbar DMA)

    nc = bacc.Bacc("TRN2", target_bir_lowering=False, debug=False)

    xT = nc.dram_tensor("xT", [dm, tok], BF16, kind="ExternalInput")
    wqT = nc.dram_tensor("wqT", [dm, hd], BF16, kind="ExternalInput")
    wkT = nc.dram_tensor("wkT", [dm, hd], BF16, kind="ExternalInput")
    wvT = nc.dram_tensor("wvT", [dm, hd], BF16, kind="ExternalInput")
    woT = nc.dram_tensor("woT", [hd, dm], BF16, kind="ExternalInput")
    bq = nc.dram_tensor("bq", [hd, 1], F32, kind="ExternalInput")
    bk = nc.dram_tensor("bk", [hd, 1], F32, kind="ExternalInput")
    bv = nc.dram_tensor("bv", [hd, 1], F32, kind="ExternalInput")
    yT = nc.dram_tensor("yT", [dm, tok], BF16, kind="ExternalOutput")
    if _DEBUG_DUMP:
        nd = min(b, 2)
        dbg = {
            name: nc.dram_tensor(f"dbg_{name}", [P, nd * w], dt,
                                 kind="ExternalOutput")
            for name, w, dt in (
                ("qT", s, BF16), ("kT", s, BF16),
                ("vT", s, BF16 if _DMA_VTRANS else F32),
                ("v65", (hd // d) * (s // P) * 80, BF16),
                ("att", 1024, BF16), ("AT", s, BF16),
                ("rec", 512, F32), ("bcs", 512, F32),
                ("reca", 512, F32),
            )
        }

    with TileContext(nc) as tc:
        with (
            nc.allow_low_precision(reason="bf16 matmul operands by design"),
            tc.tile_pool(name="const", bufs=1) as const,
            tc.tile_pool(name="xin", bufs=2) as xin,
            tc.tile_pool(name="qkv", bufs=2) as qkv,
            tc.tile_pool(name="att", bufs=3) as attp,
            tc.tile_pool(name="ATp", bufs=2) as ATp,
            tc.tile_pool(name="smal", bufs=4) as smal,
            tc.tile_pool(name="out", bufs=4) as outp,
            tc.tile_pool(name="psS", bufs=2, space="PSUM") as psS,
            tc.tile_pool(name="psO", bufs=2, space="PSUM") as psO,
            tc.tile_pool(name="psA", bufs=2, space="PSUM") as psA,
        ):
            # ---- weights / constants (resident) ----
            wq_sb = const.tile([P, dm], BF16)   # [128, kc*hd] chunks of wqT
            wk_sb = const.tile([P, dm], BF16)
            wv_sb = const.tile([P, dm], BF16)
            wo_sb = const.tile([P, dm], BF16)
            bq_sb = const.tile([hd, 1], F32)
            bk_sb = const.tile([hd, 1], F32)
            bv_sb = const.tile([hd, 1], F32)
            ones_f32 = const.tile([P, d], F32)
            nc.vector.memset(ones_f32[:], 1.0)
            ones_col = const.tile([1, d], F32R)   # bc matmul stationary
            nc.vector.tensor_copy(ones_col[:], ones_f32[0:1, :])
            ones_bf = const.tile([P, hpc * n_kt], BF16)  # v65 ones columns
            nc.vector.tensor_copy(ones_bf[:], ones_f32[:, 0:hpc * n_kt])
            if not _DMA_VTRANS:
                from concourse.masks import make_identity
                ident = const.tile([P, P], F32)
                make_identity(nc, ident[:])
            for w_sb, w_dr in ((wq_sb, wqT), (wk_sb, wkT), (wv_sb, wvT)):
                for kc in range(dmc):
                    nc.sync.dma_start(
                        w_sb[:, kc * hd:(kc + 1) * hd],
                        w_dr[kc * P:(kc + 1) * P, :],
                    )
            nc.sync.dma_start(wo_sb[:], woT[:, :])
            nc.sync.dma_start(bq_sb[:], bq[:, :])
            nc.sync.dma_start(bk_sb[:], bk[:, :])
            nc.sync.dma_start(bv_sb[:], bv[:, :])

            state = {}

            def alloc_batch(bi):
                state[bi] = {
                    "qT": qkv.tile([P, s], BF16, tag="qT", name="qT"),
                    "kT": qkv.tile([P, s], BF16, tag="kT", name="kT"),
                    "vT": qkv.tile([P, s], BF16 if _DMA_VTRANS else F32,
                                   tag="vT", name="vT"),
                    "v65": qkv.tile([P, hpc * n_kt * vstr], BF16, tag="v65",
                                    name="v65"),
                    "AT": ATp.tile([P, s], BF16, tag="AT", name="AT"),
                }

            def xload(bi, t):
                xt = xin.tile([P, dmc * 512], BF16, tag="xt")
                c0 = bi * s + t * 512
                for kc in range(dmc):
                    nc.sync.dma_start(
                        xt[:, kc * 512:(kc + 1) * 512],
                        xT[kc * P:(kc + 1) * P, c0:c0 + 512],
                    )
                state[bi]["xt", t] = xt

            _projs = ((wq_sb, bq_sb, "qT"), (wk_sb, bk_sb, "kT"),
                      (wv_sb, bv_sb, "vT"))

            def proj(bi, t, wi):
                w_sb, bias, dst = _projs[wi]
                xt = state[bi]["xt", t]
                ps = psA.tile([P, 512], F32, tag="psA")
                for kc in range(dmc):
                    nc.tensor.matmul(
                        ps[:],
                        w_sb[:, kc * hd:(kc + 1) * hd],
                        xt[:, kc * 512:(kc + 1) * 512],
                        start=(kc == 0),
                        stop=(kc == dmc - 1),
                    )
                nc.vector.tensor_scalar_add(
                    state[bi][dst][:, t * 512:(t + 1) * 512], ps[:], bias[:]
                )

            def vtrans(bi, h):
                st = state[bi]
                if _DMA_VTRANS:
                    dst = st["v65"][:, h * n_kt * vstr:(h + 1) * n_kt * vstr]
                    dst = dst.rearrange("p (c o) -> p c o", o=vstr)[:, :, 0:d]
                    nc.sync.dma_start_transpose(
                        dst, st["vT"][h * d:(h + 1) * d, :])
                    return
                hr = h * d
                for c in range(n_kt):
                    pst = psA.tile([P, 512], F32, tag="psA")
                    nc.tensor.transpose(
                        pst[:, 0:d],
                        st["vT"][hr:hr + d, c * P:(c + 1) * P],
                        ident[hr:hr + d, hr:hr + d],
                    )
                    base = (h * n_kt + c) * vstr
                    nc.vector.tensor_copy(
                        st["v65"][:, base:base + d], pst[:, 0:d])

            def vones(bi):
                st = state[bi]
                nc.vector.tensor_copy(
                    st["v65"].rearrange("p (c o) -> p c o", o=vstr)[:, :, 64],
                    ones_bf[:, 0:hpc * n_kt],
                )

            def cstep(bi, qc, kt, psos):
                st = state[bi]
                q0 = qc * 512
                pss = psS.tile([P, 1024], F32, tag="pss")
                # two heads' scores back to back: disjoint partition rows
                # (0:64 / 64:128) -> concurrent row-tiled matmuls on the PE
                for h in range(hpc):
                    nc.tensor.matmul(
                        pss[:, h * 512:(h + 1) * 512],
                        st["kT"][h * d:(h + 1) * d, kt * P:(kt + 1) * P],
                        st["qT"][h * d:(h + 1) * d, q0:q0 + 512],
                        start=True,
                        stop=True,
                    )
                att = attp.tile([P, 1024], BF16, tag="att")
                nc.scalar.activation(att[:], pss[:], AF.Exp)
                if _DEBUG_DUMP and bi < 2 and qc == 0 and kt == 0:
                    nc.sync.dma_start(
                        dbg["att"][:, bi * 1024:(bi + 1) * 1024], att[:])
                for h in range(hpc):
                    vb = (h * n_kt + kt) * vstr
                    nc.tensor.matmul(
                        psos[h][:],
                        st["v65"][:, vb:vb + 65],
                        att[:, h * 512:(h + 1) * 512],
                        start=(kt == 0),
                        stop=(kt == n_kt - 1),
                    )

            def norm(bi, qc, h, pso):
                st = state[bi]
                sums_sb = smal.tile([1, 512], F32, tag="sums")
                nc.vector.tensor_copy(sums_sb[:], pso[64:65, :])
                rec = smal.tile([1, 512], F32, tag="rec")
                nc.vector.reciprocal_approx_fast(out=rec[:], in_=sums_sb[:])
                rec_r = smal.tile([1, 512], F32R, tag="recr")
                nc.vector.tensor_copy(rec_r[:], rec[:])
                if _DEBUG_DUMP and bi == 0 and qc == 0:
                    nc.sync.dma_start(
                        dbg["reca"][0:1, h * 512:(h + 1) * 512], rec[:])
                bc = psA.tile([d, 512], F32, tag="psA")
                nc.tensor.matmul(
                    bc[:], ones_col[:], rec_r[:],
                    start=True, stop=True,
                )
                bcs = smal.tile([d, 512], F32, tag="bcs")
                nc.vector.tensor_copy(bcs[:], bc[:])
                nc.vector.tensor_mul(
                    st["AT"][h * d:(h + 1) * d, qc * 512:(qc + 1) * 512],
                    pso[0:d, :],
                    bcs[:],
                )
                if _DEBUG_DUMP and bi == 0 and qc == 0:
                    nc.sync.dma_start(
                        dbg["rec"][0:1, h * 512:(h + 1) * 512], rec[:])
                    nc.sync.dma_start(
                        dbg["bcs"][0:d, h * 512:(h + 1) * 512], bcs[:])

            def dgroup(bi, qc, half):
                st = state[bi]
                t0 = bi * s + qc * 512
                for ot in range(half * 4, half * 4 + 4):
                    psy = psA.tile([P, 512], F32, tag="psA")
                    nc.tensor.matmul(
                        psy[:],
                        wo_sb[:, ot * P:(ot + 1) * P],
                        st["AT"][:, qc * 512:(qc + 1) * 512],
                        start=True,
                        stop=True,
                    )
                    yst = outp.tile([P, 512], BF16, tag="yst")
                    nc.vector.tensor_copy(yst[:], psy[:])
                    nc.sync.dma_start(
                        yT[ot * P:(ot + 1) * P, t0:t0 + 512], yst[:]
                    )

            # ---- prologue: batch 0 projections + V transpose ----
            alloc_batch(0)
            for t in range(n_tc):
                xload(0, t)
                for wi in range(3):
                    proj(0, t, wi)
            vtrans(0, 0)
            vtrans(0, 1)
            vones(0)

            # ---- main loop: attention with interleaved fillers ----
            fillers = deque()
            for bi in range(b):
                if bi + 1 < b:
                    alloc_batch(bi + 1)
                    for t in range(n_tc):
                        fillers.append(lambda bi=bi, t=t: xload(bi + 1, t))
                        for wi in range(3):
                            fillers.append(
                                lambda bi=bi, t=t, wi=wi: proj(bi + 1, t, wi)
                            )
                    fillers.append(lambda bi=bi: vtrans(bi + 1, 0))
                    fillers.append(lambda bi=bi: vtrans(bi + 1, 1))
                    fillers.append(lambda bi=bi: vones(bi + 1))
                if not _INTERLEAVE:
                    while fillers:
                        fillers.popleft()()
                if _DEBUG_DUMP and bi < 2:
                    for nm, w in (("qT", s), ("kT", s), ("vT", s),
                                  ("v65", hpc * n_kt * vstr)):
                        nc.sync.dma_start(
                            dbg[nm][:, bi * w:(bi + 1) * w], state[bi][nm][:])
                last = bi == b - 1
                for qc in range(n_qc):
                    psos = [psO.tile([65, 512], F32, tag="pso", name="pso")
                            for _ in range(hpc)]
                    for kt in range(n_kt):
                        cstep(bi, qc, kt, psos)
                        if _INTERLEAVE and fillers and (
                                kt % 2 == 1 or last or len(fillers) > 6):
                            fillers.popleft()()
                    for h in range(hpc):
                        norm(bi, qc, h, psos[h])
                    # keep the PE fed through the reciprocal/broadcast chain
                    if _INTERLEAVE and fillers:
                        fillers.popleft()()
                    fillers.append(
                        lambda bi=bi, qc=qc: dgroup(bi, qc, 0))
                    fillers.append(
                        lambda bi=bi, qc=qc: dgroup(bi, qc, 1))
                if _DEBUG_DUMP and bi < 2:
                    nc.sync.dma_start(
                        dbg["AT"][:, bi * s:(bi + 1) * s], state[bi]["AT"][:])
                # the next batch's C-phase depends on every queued filler
                # except its own D groups; drain before crossing batches
                while fillers:
                    fillers.popleft()()
    nc.compile()
    return nc


_NC_CACHE = {}


def _get_nc(b, s):
    key = (b, s)
    if key not in _NC_CACHE:
        _NC_CACHE[key] = build_mha(b=b, s=s)
    return _NC_CACHE[key]


def kernel(inputs, Wq, bq, Wk, bk, Wv, bv, Wo, bo):
    inputs = np.asarray(inputs, dtype=np.float32)
    Wq, bq = np.asarray(Wq, np.float32), np.asarray(bq, np.float32)
    Wk, bk = np.asarray(Wk, np.float32), np.asarray(bk, np.float32)
    Wv, bv = np.asarray(Wv, np.float32), np.asarray(bv, np.float32)
    Wo, bo = np.asarray(Wo, np.float32), np.asarray(bo, np.float32)

    b, s, dm = inputs.shape
    tok = b * s
    scale = float(D_HEAD) ** 0.25

    xT = np.ascontiguousarray(inputs.reshape(tok, dm).T).astype(BF16_NP)

    in_maps = []
    for c in range(N_CORES):
        sl = slice(c * HD, (c + 1) * HD)
        in_maps.append({
            "xT": xT,
            "wqT": np.ascontiguousarray((Wq[sl, :] / scale).T).astype(BF16_NP),
            "wkT": np.ascontiguousarray((Wk[sl, :] / scale).T).astype(BF16_NP),
            "wvT": np.ascontiguousarray(Wv[sl, :].T).astype(BF16_NP),
            "woT": np.ascontiguousarray(Wo[:, sl].T).astype(BF16_NP),
            "bq": np.ascontiguousarray((bq[sl] / scale).reshape(HD, 1)),
            "bk": np.ascontiguousarray((bk[sl] / scale).reshape(HD, 1)),
            "bv": np.ascontiguousarray(bv[sl].reshape(HD, 1)),
        })

    nc = _get_nc(b, s)
    res = run_bass_kernel_spmd(
        nc, in_maps, core_ids=list(range(N_CORES)), trace=_TRACE
    )
    acc = res.results[0]["yT"].astype(np.float32)
    for c in range(1, N_CORES):
        acc += res.results[c]["yT"].astype(np.float32)
    out = acc.T.astype(np.float32) + bo[None, :]
    if _TRACE:
        kernel.last_results = res
    return out.reshape(b, s, dm)


# revision 24
# speedup vs baseline: 1.0487x; 1.0487x over previous
"""Multi-head attention (B=4, S=2048, d_model=1024, 16 heads x 64) on 8 trn2 cores.

Sharding: tensor-parallel over heads -- each core owns 2 heads (128 of the
1024 q/k/v dims and 128 columns of Wo's input dim). Each core computes a
partial output projection yT_c [1024, 8192]; the host sums the 8 partials,
adds bo, and transposes back to [4, 2048, 1024].

v2 redesign vs the fp32r baseline (977 us):
- All matmul operands bf16: separate LDWEIGHTS (background-buffer
  pull-ahead + automatic FWL for 128-col weights) instead of fp32r's
  self-loading matmuls that serialize weight load with compute.
- The two heads' score matmuls (contraction 64) are issued back to back
  on disjoint partition halves, so the PE runs them concurrently via
  row-group tiling (base_partition 0 / 64 -> tile_position rows 0 / 64).
- V transposed by DMA (dma_start_transpose) instead of PE+DVE.
- Next batch's q/k/v projections and previous output-projection chunks
  are interleaved into the attention loop as fillers, keeping the PE
  continuously busy so the HAM clock gate stays at 2.4 GHz.
- Softmax normalization per 512-q chunk: denominators ride along the AV
  matmul via a ones column in V; reciprocal_approx_fast (5x cheaper than
  DVE reciprocal), rank-1 PE broadcast, normalize fused into the
  PSUM->SBUF evacuation of the attention output.
"""

from collections import deque

import numpy as np
import ml_dtypes

import concourse.bass as bass
import concourse.mybir as mybir
from concourse import bacc
from concourse.tile import TileContext
from concourse.bass_utils import run_bass_kernel_spmd

N_HEAD = 16
D_HEAD = 64
D_MODEL = N_HEAD * D_HEAD  # 1024
B, S = 4, 2048
N_CORES = 8
HPC = N_HEAD // N_CORES  # heads per core = 2
HD = HPC * D_HEAD        # per-core head dims = 128

F32 = mybir.dt.float32
F32R = mybir.dt.float32r
BF16 = mybir.dt.bfloat16
AF = mybir.ActivationFunctionType
BF16_NP = ml_dtypes.bfloat16

_TRACE = False  # test harness can flip this for profiling
_DMA_VTRANS = True  # False: PE-transpose fallback for the V relayout
_INTERLEAVE = True  # False: run fillers en-bloc at batch boundaries
_DEBUG_DUMP = False  # True: DMA per-stage intermediates for batches 0/1


def build_mha(b=B, s=S, dm=D_MODEL, hd=HD, d=D_HEAD):
    """Build the per-core Bass program (SPMD; all cores run this)."""
    P = 128
    tok = b * s
    dmc = dm // P            # contraction chunks for projections (8)
    n_tc = s // 512          # 512-token proj chunks per batch (4)
    n_kt = s // P            # 128-token k tiles per batch (16)
    n_qc = s // 512          # 512-token q chunks per batch (4)
    hpc = hd // d            # heads per core (2)
    vstr = 80                # v65 chunk stride (160B, 32B-aligned for xbar DMA)

    nc = bacc.Bacc("TRN2", target_bir_lowering=False, debug=False)

    xT = nc.dram_tensor("xT", [dm, tok], BF16, kind="ExternalInput")
    wqT = nc.dram_tensor("wqT", [dm, hd], BF16, kind="ExternalInput")
    wkT = nc.dram_tensor("wkT", [dm, hd], BF16, kind="ExternalInput")
    wvT = nc.dram_tensor("wvT", [dm, hd], BF16, kind="ExternalInput")
    woT = nc.dram_tensor("woT", [hd, dm], BF16, kind="ExternalInput")
    bq = nc.dram_tensor("bq", [hd, 1], F32, kind="ExternalInput")
    bk = nc.dram_tensor("bk", [hd, 1], F32, kind="ExternalInput")
    bv = nc.dram_tensor("bv", [hd, 1], F32, kind="ExternalInput")
    yT = nc.dram_tensor("yT", [dm, tok], BF16, kind="ExternalOutput")
    if _DEBUG_DUMP:
        nd = min(b, 2)
        dbg = {
            name: nc.dram_tensor(f"dbg_{name}", [P, nd * w], dt,
                                 kind="ExternalOutput")
            for name, w, dt in (
                ("qT", s, BF16), ("kT", s, BF16),
                ("vT", s, BF16 if _DMA_VTRANS else F32),
                ("v65", (hd // d) * (s // P) * 80, BF16),
                ("att", 1024, BF16), ("AT", s, BF16),
                ("rec", 512, F32), ("bcs", 512, F32),
            )
        }

    with TileContext(nc) as tc:
        with (
            nc.allow_low_precision(reason="bf16 matmul operands by design"),
            tc.tile_pool(name="const", bufs=1) as const,
            tc.tile_pool(name="xin", bufs=2) as xin,
            tc.tile_pool(name="qkv", bufs=2) as qkv,
            tc.tile_pool(name="att", bufs=3) as attp,
            tc.tile_pool(name="ATp", bufs=2) as ATp,
            tc.tile_pool(name="smal", bufs=4) as smal,
            tc.tile_pool(name="out", bufs=4) as outp,
            tc.tile_pool(name="psS", bufs=2, space="PSUM") as psS,
            tc.tile_pool(name="psO", bufs=2, space="PSUM") as psO,
            tc.tile_pool(name="psA", bufs=2, space="PSUM") as psA,
        ):
            # ---- weights / constants (resident) ----
            wq_sb = const.tile([P, dm], BF16)   # [128, kc*hd] chunks of wqT
            wk_sb = const.tile([P, dm], BF16)
            wv_sb = const.tile([P, dm], BF16)
            wo_sb = const.tile([P, dm], BF16)
            bq_sb = const.tile([hd, 1], F32)
            bk_sb = const.tile([hd, 1], F32)
            bv_sb = const.tile([hd, 1], F32)
            ones_f32 = const.tile([P, d], F32)
            nc.vector.memset(ones_f32[:], 1.0)
            ones_col = const.tile([1, d], F32R)   # bc matmul stationary
            nc.vector.tensor_copy(ones_col[:], ones_f32[0:1, :])
            ones_bf = const.tile([P, hpc * n_kt], BF16)  # v65 ones columns
            nc.vector.tensor_copy(ones_bf[:], ones_f32[:, 0:hpc * n_kt])
            if not _DMA_VTRANS:
                from concourse.masks import make_identity
                ident = const.tile([P, P], F32)
                make_identity(nc, ident[:])
            nc.sync.dma_start(bq_sb[:], bq[:, :])
            nc.sync.dma_start(bk_sb[:], bk[:, :])
            nc.sync.dma_start(bv_sb[:], bv[:, :])

            def wload(w_sb, w_dr):
                for kc in range(dmc):
                    nc.sync.dma_start(
                        w_sb[:, kc * hd:(kc + 1) * hd],
                        w_dr[kc * P:(kc + 1) * P, :],
                    )

            state = {}

            def alloc_batch(bi):
                state[bi] = {
                    "qT": qkv.tile([P, s], BF16, tag="qT", name="qT"),
                    "kT": qkv.tile([P, s], BF16, tag="kT", name="kT"),
                    "vT": qkv.tile([P, s], BF16 if _DMA_VTRANS else F32,
                                   tag="vT", name="vT"),
                    "v65": qkv.tile([P, hpc * n_kt * vstr], BF16, tag="v65",
                                    name="v65"),
                    "AT": ATp.tile([P, s], BF16, tag="AT", name="AT"),
                }

            def xload(bi, t):
                xt = xin.tile([P, dmc * 512], BF16, tag="xt")
                c0 = bi * s + t * 512
                for kc in range(dmc):
                    nc.sync.dma_start(
                        xt[:, kc * 512:(kc + 1) * 512],
                        xT[kc * P:(kc + 1) * P, c0:c0 + 512],
                    )
                state[bi]["xt", t] = xt

            _projs = ((wq_sb, bq_sb, "qT"), (wk_sb, bk_sb, "kT"),
                      (wv_sb, bv_sb, "vT"))

            def proj(bi, t, wi):
                w_sb, bias, dst = _projs[wi]
                xt = state[bi]["xt", t]
                ps = psA.tile([P, 512], F32, tag="psA")
                for kc in range(dmc):
                    nc.tensor.matmul(
                        ps[:],
                        w_sb[:, kc * hd:(kc + 1) * hd],
                        xt[:, kc * 512:(kc + 1) * 512],
                        start=(kc == 0),
                        stop=(kc == dmc - 1),
                    )
                nc.vector.tensor_scalar_add(
                    state[bi][dst][:, t * 512:(t + 1) * 512], ps[:], bias[:]
                )

            def vtrans(bi, h):
                st = state[bi]
                if _DMA_VTRANS:
                    dst = st["v65"][:, h * n_kt * vstr:(h + 1) * n_kt * vstr]
                    dst = dst.rearrange("p (c o) -> p c o", o=vstr)[:, :, 0:d]
                    nc.sync.dma_start_transpose(
                        dst, st["vT"][h * d:(h + 1) * d, :])
                    return
                hr = h * d
                for c in range(n_kt):
                    pst = psA.tile([P, 512], F32, tag="psA")
                    nc.tensor.transpose(
                        pst[:, 0:d],
                        st["vT"][hr:hr + d, c * P:(c + 1) * P],
                        ident[hr:hr + d, hr:hr + d],
                    )
                    base = (h * n_kt + c) * vstr
                    nc.vector.tensor_copy(
                        st["v65"][:, base:base + d], pst[:, 0:d])

            def vones(bi):
                st = state[bi]
                nc.vector.tensor_copy(
                    st["v65"].rearrange("p (c o) -> p c o", o=vstr)[:, :, 64],
                    ones_bf[:, 0:hpc * n_kt],
                )

            def cstep_scores(bi, qc, kt):
                st = state[bi]
                q0 = qc * 512
                pss = psS.tile([P, 1024], F32, tag="pss")
                # two heads' scores back to back: disjoint partition rows
                # (0:64 / 64:128) -> concurrent row-tiled matmuls on the PE
                for h in range(hpc):
                    nc.tensor.matmul(
                        pss[:, h * 512:(h + 1) * 512],
                        st["kT"][h * d:(h + 1) * d, kt * P:(kt + 1) * P],
                        st["qT"][h * d:(h + 1) * d, q0:q0 + 512],
                        start=True,
                        stop=True,
                    )
                att = attp.tile([P, 1024], BF16, tag="att")
                nc.scalar.activation(att[:], pss[:], AF.Exp)
                if _DEBUG_DUMP and bi < 2 and qc == 0 and kt == 0:
                    nc.sync.dma_start(
                        dbg["att"][:, bi * 1024:(bi + 1) * 1024], att[:])
                return att

            def cstep_av(bi, qc, kt, psos, att):
                st = state[bi]
                for h in range(hpc):
                    vb = (h * n_kt + kt) * vstr
                    nc.tensor.matmul(
                        psos[h][:],
                        st["v65"][:, vb:vb + 65],
                        att[:, h * 512:(h + 1) * 512],
                        start=(kt == 0),
                        stop=(kt == n_kt - 1),
                    )

            def norm_recip(pso):
                # DVE-only prefix of the normalization; no tensor instr
                sums_sb = smal.tile([1, 512], F32, tag="sums")
                nc.vector.tensor_copy(sums_sb[:], pso[64:65, :])
                rec = smal.tile([1, 512], F32, tag="rec")
                nc.vector.reciprocal_approx_fast(out=rec[:], in_=sums_sb[:])
                rec_r = smal.tile([1, 512], F32R, tag="recr")
                nc.vector.tensor_copy(rec_r[:], rec[:])
                return rec_r

            def norm_apply(bi, qc, h, pso, rec_r):
                st = state[bi]
                bc = psA.tile([d, 512], F32, tag="psA")
                nc.tensor.matmul(
                    bc[:], ones_col[:], rec_r[:],
                    start=True, stop=True,
                )
                bcs = smal.tile([d, 512], F32, tag="bcs")
                nc.vector.tensor_copy(bcs[:], bc[:])
                nc.vector.tensor_mul(
                    st["AT"][h * d:(h + 1) * d, qc * 512:(qc + 1) * 512],
                    pso[0:d, :],
                    bcs[:],
                )

            def dgroup(bi, qc, half):
                st = state[bi]
                t0 = bi * s + qc * 512
                for ot in range(half * 4, half * 4 + 4):
                    psy = psA.tile([P, 512], F32, tag="psA")
                    nc.tensor.matmul(
                        psy[:],
                        wo_sb[:, ot * P:(ot + 1) * P],
                        st["AT"][:, qc * 512:(qc + 1) * 512],
                        start=True,
                        stop=True,
                    )
                    yst = outp.tile([P, 512], BF16, tag="yst")
                    nc.vector.tensor_copy(yst[:], psy[:])
                    nc.sync.dma_start(
                        yT[ot * P:(ot + 1) * P, t0:t0 + 512], yst[:]
                    )

            # ---- prologue: batch 0 projections + V transpose ----
            alloc_batch(0)
            wload(wq_sb, wqT)
            xload(0, 0)
            proj(0, 0, 0)
            wload(wk_sb, wkT)
            proj(0, 0, 1)
            wload(wv_sb, wvT)
            proj(0, 0, 2)
            nc.sync.dma_start(wo_sb[:], woT[:, :])
            for t in range(1, n_tc):
                xload(0, t)
                for wi in range(3):
                    proj(0, t, wi)
            vtrans(0, 0)
            vtrans(0, 1)
            vones(0)

            # ---- main loop: attention with interleaved fillers ----
            fillers = deque()
            pending = None
            for bi in range(b):
                if bi + 1 < b:
                    alloc_batch(bi + 1)
                    for t in range(n_tc):
                        fillers.append(lambda bi=bi, t=t: xload(bi + 1, t))
                        for wi in range(3):
                            fillers.append(
                                lambda bi=bi, t=t, wi=wi: proj(bi + 1, t, wi)
                            )
                    fillers.append(lambda bi=bi: vtrans(bi + 1, 0))
                    fillers.append(lambda bi=bi: vtrans(bi + 1, 1))
                    fillers.append(lambda bi=bi: vones(bi + 1))
                if not _INTERLEAVE:
                    while fillers:
                        fillers.popleft()()
                if _DEBUG_DUMP and bi < 2:
                    for nm, w in (("qT", s), ("kT", s), ("vT", s),
                                  ("v65", hpc * n_kt * vstr)):
                        nc.sync.dma_start(
                            dbg[nm][:, bi * w:(bi + 1) * w], state[bi][nm][:])
                for qc in range(n_qc):
                    psos = [psO.tile([65, 512], F32, tag="pso", name="pso")
                            for _ in range(hpc)]
                    for kt in range(n_kt):
                        att = cstep_scores(bi, qc, kt)
                        if kt == 0 and pending is not None:
                            # previous qc's bc matmuls land here, behind
                            # already-issued scores whose rec_r is ready
                            pbi, pqc, items = pending
                            for h, pso, rr in items:
                                norm_apply(pbi, pqc, h, pso, rr)
                            fillers.append(lambda bi=pbi, qc=pqc:
                                           dgroup(bi, qc, 0))
                            fillers.append(lambda bi=pbi, qc=pqc:
                                           dgroup(bi, qc, 1))
                            pending = None
                        cstep_av(bi, qc, kt, psos, att)
                        if _INTERLEAVE and fillers and kt % 2 == 1:
                            fillers.popleft()()
                    pending = (bi, qc, [(h, psos[h], norm_recip(psos[h]))
                                        for h in range(hpc)])
                if _DEBUG_DUMP and bi < 2:
                    nc.sync.dma_start(
                        dbg["AT"][:, bi * s:(bi + 1) * s], state[bi]["AT"][:])
                # drain when in-loop pops cannot keep up (small configs) or
                # at the very end; otherwise let leftovers flow into the
                # next batch's pop slots
                if bi == b - 1 or n_qc * n_kt // 2 < 30 or not _INTERLEAVE:
                    if pending is not None:
                        pbi, pqc, items = pending
                        for h, pso, rr in items:
                            norm_apply(pbi, pqc, h, pso, rr)
                        fillers.append(lambda bi=pbi, qc=pqc: dgroup(bi, qc, 0))
                        fillers.append(lambda bi=pbi, qc=pqc: dgroup(bi, qc, 1))
                        pending = None
                    while fillers:
                        fillers.popleft()()
    nc.compile()
    return nc


_NC_CACHE = {}


def _get_nc(b, s):
    key = (b, s)
    if key not in _NC_CACHE:
        _NC_CACHE[key] = build_mha(b=b, s=s)
    return _NC_CACHE[key]


def kernel(inputs, Wq, bq, Wk, bk, Wv, bv, Wo, bo):
    inputs = np.asarray(inputs, dtype=np.float32)
    Wq, bq = np.asarray(Wq, np.float32), np.asarray(bq, np.float32)
    Wk, bk = np.asarray(Wk, np.float32), np.asarray(bk, np.float32)
    Wv, bv = np.asarray(Wv, np.float32), np.asarray(bv, np.float32)
    Wo, bo = np.asarray(Wo, np.float32), np.asarray(bo, np.float32)

    b, s, dm = inputs.shape
    tok = b * s
    scale = float(D_HEAD) ** 0.25

    xT = np.ascontiguousarray(inputs.reshape(tok, dm).T).astype(BF16_NP)

    in_maps = []
    for c in range(N_CORES):
        sl = slice(c * HD, (c + 1) * HD)
        in_maps.append({
            "xT": xT,
            "wqT": np.ascontiguousarray((Wq[sl, :] / scale).T).astype(BF16_NP),
            "wkT": np.ascontiguousarray((Wk[sl, :] / scale).T).astype(BF16_NP),
            "wvT": np.ascontiguousarray(Wv[sl, :].T).astype(BF16_NP),
            "woT": np.ascontiguousarray(Wo[:, sl].T).astype(BF16_NP),
            "bq": np.ascontiguousarray((bq[sl] / scale).reshape(HD, 1)),
            "bk": np.ascontiguousarray((bk[sl] / scale).reshape(HD, 1)),
            "bv": np.ascontiguousarray(bv[sl].reshape(HD, 1)),
        })

    nc = _get_nc(b, s)
    res = run_bass_kernel_spmd(
        nc, in_maps, core_ids=list(range(N_CORES)), trace=_TRACE
    )
    acc = res.results[0]["yT"].astype(np.float32)
    for c in range(1, N_CORES):
        acc += res.results[c]["yT"].astype(np.float32)
    out = acc.T.astype(np.float32) + bo[None, :]
    if _TRACE:
        kernel.last_results = res
    return out.reshape(b, s, dm)


# revision 25
# speedup vs baseline: 1.0628x; 1.0135x over previous
"""Multi-head attention (B=4, S=2048, d_model=1024, 16 heads x 64) on 8 trn2 cores.

Sharding: tensor-parallel over heads -- each core owns 2 heads (128 of the
1024 q/k/v dims and 128 columns of Wo's input dim). Each core computes a
partial output projection yT_c [1024, 8192]; the host sums the 8 partials,
adds bo, and transposes back to [4, 2048, 1024].

v2 redesign vs the fp32r baseline (977 us):
- All matmul operands bf16: separate LDWEIGHTS (background-buffer
  pull-ahead + automatic FWL for 128-col weights) instead of fp32r's
  self-loading matmuls that serialize weight load with compute.
- The two heads' score matmuls (contraction 64) are issued back to back
  on disjoint partition halves, so the PE runs them concurrently via
  row-group tiling (base_partition 0 / 64 -> tile_position rows 0 / 64).
- V transposed by DMA (dma_start_transpose) instead of PE+DVE.
- Next batch's q/k/v projections and previous output-projection chunks
  are interleaved into the attention loop as fillers, keeping the PE
  continuously busy so the HAM clock gate stays at 2.4 GHz.
- Softmax normalization per 512-q chunk: denominators ride along the AV
  matmul via a ones column in V; reciprocal_approx_fast (5x cheaper than
  DVE reciprocal), rank-1 PE broadcast, normalize fused into the
  PSUM->SBUF evacuation of the attention output.
"""

from collections import deque

import numpy as np
import ml_dtypes

import concourse.bass as bass
import concourse.mybir as mybir
from concourse import bacc
from concourse.tile import TileContext
from concourse.bass_utils import run_bass_kernel_spmd

N_HEAD = 16
D_HEAD = 64
D_MODEL = N_HEAD * D_HEAD  # 1024
B, S = 4, 2048
N_CORES = 8
HPC = N_HEAD // N_CORES  # heads per core = 2
HD = HPC * D_HEAD        # per-core head dims = 128

F32 = mybir.dt.float32
F32R = mybir.dt.float32r
BF16 = mybir.dt.bfloat16
AF = mybir.ActivationFunctionType
BF16_NP = ml_dtypes.bfloat16

_TRACE = False  # test harness can flip this for profiling
_DMA_VTRANS = True  # False: PE-transpose fallback for the V relayout
_INTERLEAVE = True  # False: run fillers en-bloc at batch boundaries
_DEBUG_DUMP = False  # True: DMA per-stage intermediates for batches 0/1


def build_mha(b=B, s=S, dm=D_MODEL, hd=HD, d=D_HEAD):
    """Build the per-core Bass program (SPMD; all cores run this)."""
    P = 128
    tok = b * s
    dmc = dm // P            # contraction chunks for projections (8)
    n_tc = s // 512          # 512-token proj chunks per batch (4)
    n_kt = s // P            # 128-token k tiles per batch (16)
    n_qc = s // 512          # 512-token q chunks per batch (4)
    hpc = hd // d            # heads per core (2)
    vstr = 80                # v65 chunk stride (160B, 32B-aligned for xbar DMA)

    nc = bacc.Bacc("TRN2", target_bir_lowering=False, debug=False)

    xT = nc.dram_tensor("xT", [dm, tok], BF16, kind="ExternalInput")
    wqT = nc.dram_tensor("wqT", [dm, hd], BF16, kind="ExternalInput")
    wkT = nc.dram_tensor("wkT", [dm, hd], BF16, kind="ExternalInput")
    wvT = nc.dram_tensor("wvT", [dm, hd], BF16, kind="ExternalInput")
    woT = nc.dram_tensor("woT", [hd, dm], BF16, kind="ExternalInput")
    bq = nc.dram_tensor("bq", [hd, 1], F32, kind="ExternalInput")
    bk = nc.dram_tensor("bk", [hd, 1], F32, kind="ExternalInput")
    bv = nc.dram_tensor("bv", [hd, 1], F32, kind="ExternalInput")
    yT = nc.dram_tensor("yT", [dm, tok], BF16, kind="ExternalOutput")
    if _DEBUG_DUMP:
        nd = min(b, 2)
        dbg = {
            name: nc.dram_tensor(f"dbg_{name}", [P, nd * w], dt,
                                 kind="ExternalOutput")
            for name, w, dt in (
                ("qT", s, BF16), ("kT", s, BF16),
                ("vT", s, BF16 if _DMA_VTRANS else F32),
                ("v65", (hd // d) * (s // P) * 80, BF16),
                ("att", 1024, BF16), ("AT", s, BF16),
                ("rec", 512, F32), ("bcs", 512, F32),
            )
        }

    with TileContext(nc) as tc:
        with (
            nc.allow_low_precision(reason="bf16 matmul operands by design"),
            tc.tile_pool(name="const", bufs=1) as const,
            tc.tile_pool(name="xin", bufs=2) as xin,
            tc.tile_pool(name="qkv", bufs=2) as qkv,
            tc.tile_pool(name="att", bufs=3) as attp,
            tc.tile_pool(name="ATp", bufs=2) as ATp,
            tc.tile_pool(name="smal", bufs=4) as smal,
            tc.tile_pool(name="out", bufs=4) as outp,
            tc.tile_pool(name="psS", bufs=2, space="PSUM") as psS,
            tc.tile_pool(name="psO", bufs=2, space="PSUM") as psO,
            tc.tile_pool(name="psA", bufs=2, space="PSUM") as psA,
        ):
            # ---- weights / constants (resident) ----
            wq_sb = const.tile([P, dm], BF16)   # [128, kc*hd] chunks of wqT
            wk_sb = const.tile([P, dm], BF16)
            wv_sb = const.tile([P, dm], BF16)
            wo_sb = const.tile([P, dm], BF16)
            bq_sb = const.tile([hd, 1], F32)
            bk_sb = const.tile([hd, 1], F32)
            bv_sb = const.tile([hd, 1], F32)
            ones_f32 = const.tile([P, d], F32)
            nc.vector.memset(ones_f32[:], 1.0)
            ones_col = const.tile([1, d], F32R)   # bc matmul stationary
            nc.vector.tensor_copy(ones_col[:], ones_f32[0:1, :])
            ones_bf = const.tile([P, hpc * n_kt], BF16)  # v65 ones columns
            nc.vector.tensor_copy(ones_bf[:], ones_f32[:, 0:hpc * n_kt])
            if not _DMA_VTRANS:
                from concourse.masks import make_identity
                ident = const.tile([P, P], F32)
                make_identity(nc, ident[:])
            nc.sync.dma_start(bq_sb[:], bq[:, :])
            nc.sync.dma_start(bk_sb[:], bk[:, :])
            nc.sync.dma_start(bv_sb[:], bv[:, :])

            def wload(w_sb, w_dr):
                for kc in range(dmc):
                    nc.sync.dma_start(
                        w_sb[:, kc * hd:(kc + 1) * hd],
                        w_dr[kc * P:(kc + 1) * P, :],
                    )

            state = {}

            def alloc_batch(bi):
                state[bi] = {
                    "qT": qkv.tile([P, s], BF16, tag="qT", name="qT"),
                    "kT": qkv.tile([P, s], BF16, tag="kT", name="kT"),
                    "vT": qkv.tile([P, s], BF16 if _DMA_VTRANS else F32,
                                   tag="vT", name="vT"),
                    "v65": qkv.tile([P, hpc * n_kt * vstr], BF16, tag="v65",
                                    name="v65"),
                    "AT": ATp.tile([P, s], BF16, tag="AT", name="AT"),
                }

            def xload(bi, t):
                xt = xin.tile([P, dmc * 512], BF16, tag="xt")
                c0 = bi * s + t * 512
                for kc in range(dmc):
                    nc.sync.dma_start(
                        xt[:, kc * 512:(kc + 1) * 512],
                        xT[kc * P:(kc + 1) * P, c0:c0 + 512],
                    )
                state[bi]["xt", t] = xt

            _projs = ((wq_sb, bq_sb, "qT"), (wk_sb, bk_sb, "kT"),
                      (wv_sb, bv_sb, "vT"))

            def proj(bi, t, wi):
                w_sb, bias, dst = _projs[wi]
                xt = state[bi]["xt", t]
                ps = psA.tile([P, 512], F32, tag="psA")
                for kc in range(dmc):
                    nc.tensor.matmul(
                        ps[:],
                        w_sb[:, kc * hd:(kc + 1) * hd],
                        xt[:, kc * 512:(kc + 1) * 512],
                        start=(kc == 0),
                        stop=(kc == dmc - 1),
                    )
                nc.vector.tensor_scalar_add(
                    state[bi][dst][:, t * 512:(t + 1) * 512], ps[:], bias[:]
                )

            def vtrans(bi, h, t=None):
                st = state[bi]
                ts = range(n_tc) if t is None else (t,)
                if _DMA_VTRANS:
                    cpt = n_kt // n_tc  # v65 chunks per 512-token slice (4)
                    for ti in ts:
                        c0 = (h * n_kt + ti * cpt) * vstr
                        dst = st["v65"][:, c0:c0 + cpt * vstr]
                        dst = dst.rearrange(
                            "p (c o) -> p c o", o=vstr)[:, :, 0:d]
                        nc.sync.dma_start_transpose(
                            dst,
                            st["vT"][h * d:(h + 1) * d,
                                     ti * 512:(ti + 1) * 512])
                    return
                hr = h * d
                for c in range(n_kt):
                    pst = psA.tile([P, 512], F32, tag="psA")
                    nc.tensor.transpose(
                        pst[:, 0:d],
                        st["vT"][hr:hr + d, c * P:(c + 1) * P],
                        ident[hr:hr + d, hr:hr + d],
                    )
                    base = (h * n_kt + c) * vstr
                    nc.vector.tensor_copy(
                        st["v65"][:, base:base + d], pst[:, 0:d])

            def vones(bi):
                st = state[bi]
                nc.vector.tensor_copy(
                    st["v65"].rearrange("p (c o) -> p c o", o=vstr)[:, :, 64],
                    ones_bf[:, 0:hpc * n_kt],
                )

            def cstep_scores(bi, qc, kt):
                st = state[bi]
                q0 = qc * 512
                pss = psS.tile([P, 1024], F32, tag="pss")
                # two heads' scores back to back: disjoint partition rows
                # (0:64 / 64:128) -> concurrent row-tiled matmuls on the PE
                for h in range(hpc):
                    nc.tensor.matmul(
                        pss[:, h * 512:(h + 1) * 512],
                        st["kT"][h * d:(h + 1) * d, kt * P:(kt + 1) * P],
                        st["qT"][h * d:(h + 1) * d, q0:q0 + 512],
                        start=True,
                        stop=True,
                    )
                att = attp.tile([P, 1024], BF16, tag="att")
                nc.scalar.activation(att[:], pss[:], AF.Exp)
                if _DEBUG_DUMP and bi < 2 and qc == 0 and kt == 0:
                    nc.sync.dma_start(
                        dbg["att"][:, bi * 1024:(bi + 1) * 1024], att[:])
                return att

            def cstep_av(bi, qc, kt, psos, att):
                st = state[bi]
                for h in range(hpc):
                    vb = (h * n_kt + kt) * vstr
                    nc.tensor.matmul(
                        psos[h][:],
                        st["v65"][:, vb:vb + 65],
                        att[:, h * 512:(h + 1) * 512],
                        start=(kt == 0),
                        stop=(kt == n_kt - 1),
                    )

            def norm_recip(pso):
                # DVE-only prefix of the normalization; no tensor instr
                sums_sb = smal.tile([1, 512], F32, tag="sums")
                nc.vector.tensor_copy(sums_sb[:], pso[64:65, :])
                rec = smal.tile([1, 512], F32, tag="rec")
                nc.vector.reciprocal_approx_fast(out=rec[:], in_=sums_sb[:])
                rec_r = smal.tile([1, 512], F32R, tag="recr")
                nc.vector.tensor_copy(rec_r[:], rec[:])
                return rec_r

            def norm_apply(bi, qc, h, pso, rec_r):
                st = state[bi]
                bc = psA.tile([d, 512], F32, tag="psA")
                nc.tensor.matmul(
                    bc[:], ones_col[:], rec_r[:],
                    start=True, stop=True,
                )
                bcs = smal.tile([d, 512], F32, tag="bcs")
                nc.vector.tensor_copy(bcs[:], bc[:])
                nc.vector.tensor_mul(
                    st["AT"][h * d:(h + 1) * d, qc * 512:(qc + 1) * 512],
                    pso[0:d, :],
                    bcs[:],
                )

            def dgroup(bi, qc, half):
                st = state[bi]
                t0 = bi * s + qc * 512
                for ot in range(half * 4, half * 4 + 4):
                    psy = psA.tile([P, 512], F32, tag="psA")
                    nc.tensor.matmul(
                        psy[:],
                        wo_sb[:, ot * P:(ot + 1) * P],
                        st["AT"][:, qc * 512:(qc + 1) * 512],
                        start=True,
                        stop=True,
                    )
                    yst = outp.tile([P, 512], BF16, tag="yst")
                    nc.vector.tensor_copy(yst[:], psy[:])
                    nc.sync.dma_start(
                        yT[ot * P:(ot + 1) * P, t0:t0 + 512], yst[:]
                    )

            # ---- prologue: batch 0 projections + V transpose ----
            alloc_batch(0)
            wload(wq_sb, wqT)
            xload(0, 0)
            proj(0, 0, 0)
            wload(wk_sb, wkT)
            proj(0, 0, 1)
            wload(wv_sb, wvT)
            vones(0)
            proj(0, 0, 2)
            vtrans(0, 0, 0)
            vtrans(0, 1, 0)
            nc.sync.dma_start(wo_sb[:], woT[:, :])
            for t in range(1, n_tc):
                xload(0, t)
                for wi in range(3):
                    proj(0, t, wi)
                vtrans(0, 0, t)
                vtrans(0, 1, t)

            # ---- main loop: attention with interleaved fillers ----
            fillers = deque()
            pending = None
            for bi in range(b):
                if bi + 1 < b:
                    alloc_batch(bi + 1)
                    fillers.append(lambda bi=bi: vones(bi + 1))
                    for t in range(n_tc):
                        fillers.append(lambda bi=bi, t=t: xload(bi + 1, t))
                        for wi in range(3):
                            fillers.append(
                                lambda bi=bi, t=t, wi=wi: proj(bi + 1, t, wi)
                            )
                        fillers.append(
                            lambda bi=bi, t=t: vtrans(bi + 1, 0, t))
                        fillers.append(
                            lambda bi=bi, t=t: vtrans(bi + 1, 1, t))
                if not _INTERLEAVE:
                    while fillers:
                        fillers.popleft()()
                if _DEBUG_DUMP and bi < 2:
                    for nm, w in (("qT", s), ("kT", s), ("vT", s),
                                  ("v65", hpc * n_kt * vstr)):
                        nc.sync.dma_start(
                            dbg[nm][:, bi * w:(bi + 1) * w], state[bi][nm][:])
                for qc in range(n_qc):
                    psos = [psO.tile([65, 512], F32, tag="pso", name="pso")
                            for _ in range(hpc)]
                    for kt in range(n_kt):
                        att = cstep_scores(bi, qc, kt)
                        if kt == 0 and pending is not None:
                            # previous qc's bc matmuls land here, behind
                            # already-issued scores whose rec_r is ready
                            pbi, pqc, items = pending
                            for h, pso, rr in items:
                                norm_apply(pbi, pqc, h, pso, rr)
                            fillers.append(lambda bi=pbi, qc=pqc:
                                           dgroup(bi, qc, 0))
                            fillers.append(lambda bi=pbi, qc=pqc:
                                           dgroup(bi, qc, 1))
                            pending = None
                        if _INTERLEAVE and fillers and kt % 2 == 1:
                            fillers.popleft()()
                        cstep_av(bi, qc, kt, psos, att)
                    pending = (bi, qc, [(h, psos[h], norm_recip(psos[h]))
                                        for h in range(hpc)])
                if _DEBUG_DUMP and bi < 2:
                    nc.sync.dma_start(
                        dbg["AT"][:, bi * s:(bi + 1) * s], state[bi]["AT"][:])
                # drain when in-loop pops cannot keep up (small configs) or
                # at the very end; otherwise let leftovers flow into the
                # next batch's pop slots
                if bi == b - 1 or n_qc * n_kt // 2 < 30 or not _INTERLEAVE:
                    if pending is not None:
                        pbi, pqc, items = pending
                        for h, pso, rr in items:
                            norm_apply(pbi, pqc, h, pso, rr)
                        fillers.append(lambda bi=pbi, qc=pqc: dgroup(bi, qc, 0))
                        fillers.append(lambda bi=pbi, qc=pqc: dgroup(bi, qc, 1))
                        pending = None
                    while fillers:
                        fillers.popleft()()
    nc.compile()
    return nc


_NC_CACHE = {}


def _get_nc(b, s):
    key = (b, s)
    if key not in _NC_CACHE:
        _NC_CACHE[key] = build_mha(b=b, s=s)
    return _NC_CACHE[key]


def kernel(inputs, Wq, bq, Wk, bk, Wv, bv, Wo, bo):
    inputs = np.asarray(inputs, dtype=np.float32)
    Wq, bq = np.asarray(Wq, np.float32), np.asarray(bq, np.float32)
    Wk, bk = np.asarray(Wk, np.float32), np.asarray(bk, np.float32)
    Wv, bv = np.asarray(Wv, np.float32), np.asarray(bv, np.float32)
    Wo, bo = np.asarray(Wo, np.float32), np.asarray(bo, np.float32)

    b, s, dm = inputs.shape
    tok = b * s
    scale = float(D_HEAD) ** 0.25

    xT = np.ascontiguousarray(inputs.reshape(tok, dm).T).astype(BF16_NP)

    in_maps = []
    for c in range(N_CORES):
        sl = slice(c * HD, (c + 1) * HD)
        in_maps.append({
            "xT": xT,
            "wqT": np.ascontiguousarray((Wq[sl, :] / scale).T).astype(BF16_NP),
            "wkT": np.ascontiguousarray((Wk[sl, :] / scale).T).astype(BF16_NP),
            "wvT": np.ascontiguousarray(Wv[sl, :].T).astype(BF16_NP),
            "woT": np.ascontiguousarray(Wo[:, sl].T).astype(BF16_NP),
            "bq": np.ascontiguousarray((bq[sl] / scale).reshape(HD, 1)),
            "bk": np.ascontiguousarray((bk[sl] / scale).reshape(HD, 1)),
            "bv": np.ascontiguousarray(bv[sl].reshape(HD, 1)),
        })

    nc = _get_nc(b, s)
    res = run_bass_kernel_spmd(
        nc, in_maps, core_ids=list(range(N_CORES)), trace=_TRACE
    )
    acc = res.results[0]["yT"].astype(np.float32)
    for c in range(1, N_CORES):
        acc += res.results[c]["yT"].astype(np.float32)
    out = acc.T.astype(np.float32) + bo[None, :]
    if _TRACE:
        kernel.last_results = res
    return out.reshape(b, s, dm)
